# revision 2
# baseline (speedup 1.0000x reference)
"""nn_BinaryConv2D Trainium2 kernel (v2).

out = conv2d(sign(x), sign(w)), 3x3, stride 1, SAME, NHWC/HWIO.
x [64, 128, 128, 64] fp32, w [3, 3, 64, 64] fp32 -> out [64, 128, 128, 64] fp32.

Sharding: data-parallel over batch across 8 NeuronCores (8 images/core);
weights replicated, pre-packed host-side into per-pass DoubleRow tap stacks.

Per-core scheme (all shapes hardcoded):
- Each image is loaded as one fully-linear [128, 8192] bf16 tile (row per
  partition, 32 KB contiguous DRAM reads) through a SWDGE DMA that casts
  fp32 -> bf16 in flight (sign preserved; DMA charged on output bytes).
- The pixel-major -> channel-major transpose runs on the PE (free-ish in
  the cost model): 63 full 128x128 is_transpose identity matmuls per image
  plus two 64-row edge transposes, staged through bf16 PSUM tiles of 8
  blocks each.  This keeps the DMA engines free for pure load/store
  traffic (the old DMA-xbar path cost ~7us/image of DMA device time and
  serialized against all other DMAs).
- ACT Sign evacuates each PSUM stage tile into the fp8 mega buffer:
  mega column (row slot s, pair c) holds [top: odd@(c-1) | bottom:
  even@c], +-1 with zero pads.  Pads are memset once per mega buffer
  (they are never overwritten, so reused buffers keep them).
- Conv = 4 fp8 DoubleRow (K=256) matmuls per PSUM group of 6 row slots
  (N=390):  DoubleRow k-tile pairs must sit at an even column distance
  (odd t-strides fault the runtime), which makes the natural 3-pass
  cover impossible (parity obstruction); the 4-pass cover pairs rows
  r-1/r+1 (t-stride 130) for both column phases, plus two row-r passes
  with a zero-weighted duplicate k-tile (t-stride 0).  2.0N cycles per
  group vs 6.0N for the non-DoubleRow floor.
- PSUM -> cm strip/cast (drop pad columns, fp32 -> fp16 exact) is split
  between DVE and ACT (Copy) to balance engine load.
- Output is stored channel-major ([img, (parity, co), row, pair]) as
  fp16 on the SP HWDGE queue and unshuffled to NHWC fp32 on the host.
"""

from contextlib import ExitStack

import numpy as np
import ml_dtypes

import concourse.bass as bass
import concourse.tile as tile
from concourse import mybir
from concourse.vector_clock import ScopedClock, VectorClock
from concourse.tile_rust import add_dep_helper

H = W = 128
C = 64
SW = 65
OFF = 1
MEGA_COLS = OFF + (H + 2) * SW + 1  # 8452
N_CORES = 8
NIMG = 8  # images per core


# ---------------------------------------------------------------------------
# Workaround for this container's walrus: CTRL instructions support only ONE
# sync-wait slot, but Tile's tail drain attaches one wait per live proc.
# Split the waits across single-wait NoOps on the SP engine (in-order), then
# drain waitless.
def _drain_and_barrier_split(self, tick_clock, wait_clock):
    nc = self.nc
    vc = tick_clock.global_clock
    n = len(vc)
    for i in range(n):
        if vc[i] > 0:
            sub = VectorClock([0] * n)
            sub.require_at_least(i, vc[i])
            nop = nc.sync.nop(nofuse=True)
            wait_clock.add_sem_waits(nop.ins, ScopedClock({None: sub}))
    nc.sync.drain()
    nc.all_engine_barrier()
    assert self.sems is not None
    popped = nc._tile_sem_poison_stack.pop()
    assert popped is self._sem_poison
    nc.clear_and_free_semaphores(list(self.sems.allocated().values()))
    nc.all_engine_barrier()


tile.TileContext._drain_and_barrier = _drain_and_barrier_split


# The same walrus limit applies to every instruction: at most one sync wait.
# Tile freely emits multi-wait instructions, so rewrite the BIR JSON right
# before compilation: hoist all but the last wait of each instruction onto
# fresh same-engine NoOps inserted immediately before it (engines execute
# their instruction stream in order, so the waits still gate the original
# instruction).
def _split_multi_waits_json(bir_bytes):
    import json as _json

    bir = _json.loads(bir_bytes)
    n = 0
    for fn in bir.get("functions", []):
        for blk in fn.get("blocks", []):
            insts = blk.get("instructions", [])
            out = []
            for ins in insts:
                si = ins.get("sync_info")
                if si:
                    waits = si.get("on_wait") or []
                    if len(waits) > 1:
                        for wv in waits[:-1]:
                            n += 1
                            out.append(
                                {
                                    "debug": ins.get("debug", 0),
                                    "engine": ins["engine"],
                                    "ins": [],
                                    "outs": [],
                                    "name": f"I-wsplit-{n}",
                                    "opcode": "NoOp",
                                    "sync_info": {
                                        "on_update": [],
                                        "on_wait": [wv],
                                    },
                                }
                            )
                        si["on_wait"] = [waits[-1]]
                out.append(ins)
            blk["instructions"] = out
    return _json.dumps(bir).encode()


def _install_compile_hook():
    from concourse import bass_utils as _bu
    from concourse import bass2jax as _b2j

    if getattr(_bu, "_orig_compile_bir_kernel", None) is None:
        _bu._orig_compile_bir_kernel = _bu.compile_bir_kernel

        def _patched(bir_json, tmpdir, neff_name="file.neff"):
            return _bu._orig_compile_bir_kernel(
                _split_multi_waits_json(bir_json), tmpdir, neff_name=neff_name
            )

        _bu.compile_bir_kernel = _patched
        _b2j.compile_bir_kernel = _patched


_install_compile_hook()
# ---------------------------------------------------------------------------


def build_nc(nimg=NIMG, gsize=6, mega_bufs=3, psum_bufs=4, io_bufs=2,
             tp_bufs=3, cm_bufs=5, gb=6, act_strip=(20, 21),
             last_act_strip=tuple(range(0, 22, 2)),
             first_load_chunks=4, steady_chunks=2):
    nc = bass.Bass()
    x = nc.dram_tensor("x", [nimg, H, W, C], mybir.dt.float32, kind="ExternalInput")
    wt = nc.dram_tensor("wt", [128, 1024], mybir.dt.float8e4, kind="ExternalInput")
    idents = nc.dram_tensor(
        "idents", [128, 128], mybir.dt.bfloat16, kind="ExternalInput"
    )
    y = nc.dram_tensor("y", [nimg, 128, H * C], mybir.dt.float16, kind="ExternalOutput")

    with tile.TileContext(nc) as tc, ExitStack() as ctx:
        wpool = ctx.enter_context(tc.tile_pool(name="wpool", bufs=1))
        mega_pool = ctx.enter_context(tc.tile_pool(name="mega", bufs=mega_bufs))
        in_pool = ctx.enter_context(tc.tile_pool(name="inp", bufs=io_bufs))
        cm_pool = ctx.enter_context(tc.tile_pool(name="cm", bufs=cm_bufs))
        tp_pool = ctx.enter_context(
            tc.tile_pool(name="tp", bufs=tp_bufs, space="PSUM")
        )
        psum_pool = ctx.enter_context(
            tc.tile_pool(name="ps", bufs=psum_bufs, space="PSUM")
        )

        wt_sb = wpool.tile([128, 1024], mybir.dt.float8e4)
        nc.sync.dma_start(out=wt_sb[:], in_=wt[:])
        id_sb = wpool.tile([128, 128], mybir.dt.bfloat16)
        nc.sync.dma_start(out=id_sb[:], in_=idents[:])

        groups = []
        r0 = 0
        while r0 < H:
            g = min(gsize, H - r0)
            groups.append((r0, g))
            r0 += g

        GB = gb
        batches = [groups[i : i + GB] for i in range(0, len(groups), GB)]
        if len(batches) >= 2 and len(batches[-1]) < GB // 2:
            batches[-2].extend(batches.pop())

        def input_load(img, chunks=1):
            # chunked variant (image 0): column-chunks let the first
            # transposes start ~4us earlier; Tile's overlap tracking gives
            # each transpose block deps on exactly the chunks it reads.
            xt = in_pool.tile([128, H * 64], mybir.dt.bfloat16)
            wc = W // chunks
            for k in range(chunks):
                nc.gpsimd.dma_start(
                    out=xt[:, k * wc * 64 : (k + 1) * wc * 64],
                    in_=x[img, :, k * wc : (k + 1) * wc].rearrange(
                        "r w c -> r (w c)"
                    ),
                )
            return xt

        def mega_setup(img):
            mega = mega_pool.tile([128, MEGA_COLS], mybir.dt.float8e4)
            if img < mega_bufs:
                # pads are zeroed once per physical buffer; Sign never
                # overwrites them, so reused buffers keep their pads
                nc.gpsimd.memset(mega[:, 0:2], 0.0)
                slots = mega[:, OFF : OFF + 130 * SW].rearrange(
                    "p (s c) -> p s c", c=SW
                )
                nc.gpsimd.memset(slots[0:64, :, 0:1], 0.0)
                nc.gpsimd.memset(slots[64:128, :, 64:65], 0.0)
                nc.gpsimd.memset(mega[:, OFF : OFF + SW], 0.0)
                b128 = OFF + 129 * SW
                nc.gpsimd.memset(mega[:, b128 : b128 + SW], 0.0)
            return mega

        def transpose_tile(img, mega, xt, q):
            # PE transposes for stage tile q: blocks 8q..8q+7 (tile 7 holds
            # 7 blocks plus the two 64-row edge transposes), then ACT Sign
            # evacuates bf16 PSUM -> fp8 mega with slot-skewed 3D APs.
            tp = tp_pool.tile([128, 1024], mybir.dt.bfloat16)
            nblk = 8 if q < 7 else 7
            for j in range(nblk):
                g = 8 * q + j
                nc.tensor.transpose(
                    tp[:, j * 128 : (j + 1) * 128],
                    xt[:, 64 + 128 * g : 192 + 128 * g],
                    id_sb[:],
                )
            if q == 7:
                # odd@63 (pixel 127) and even@0 (pixel 0) channel transposes
                nc.tensor.transpose(tp[0:64, 896:1024], xt[:, 8128:8192], id_sb[:])
                nc.tensor.transpose(tp[64:128, 896:1024], xt[:, 0:64], id_sb[:])

            dsl = mega[:, OFF + SW + 8 * q + 1 : OFF + SW + 8 * q + 2]
            dst = bass.AP(
                dsl.tensor, dsl.offset, [list(dsl.ap[0]), [1, nblk], [SW, H]]
            )
            ssl = tp[:, 0:1]
            src = bass.AP(
                ssl.tensor, ssl.offset, [list(ssl.ap[0]), [128, nblk], [1, H]]
            )
            nc.scalar.activation(dst, src, mybir.ActivationFunctionType.Sign)
            if q == 7:
                # odd@63 -> top of col 64 of each row slot
                dsl2 = mega[0:64, OFF + SW + 64 : OFF + SW + 65]
                dst2 = bass.AP(dsl2.tensor, dsl2.offset, [list(dsl2.ap[0]), [SW, H]])
                ssl2 = tp[0:64, 896:897]
                src2 = bass.AP(ssl2.tensor, ssl2.offset, [list(ssl2.ap[0]), [1, H]])
                nc.scalar.activation(dst2, src2, mybir.ActivationFunctionType.Sign)
                # even@0 -> bottom of col 0 of each row slot
                dsl3 = mega[64:128, OFF + SW : OFF + SW + 1]
                dst3 = bass.AP(dsl3.tensor, dsl3.offset, [list(dsl3.ap[0]), [SW, H]])
                ssl3 = tp[64:128, 896:897]
                src3 = bass.AP(ssl3.tensor, ssl3.offset, [list(ssl3.ap[0]), [1, H]])
                nc.scalar.activation(dst3, src3, mybir.ActivationFunctionType.Sign)

        def conv_group(img, mega, r0, g, cm, cmoff, use_act):
            N = g * SW
            ps = psum_pool.tile([128, N], mybir.dt.float32, padded_shape=[128, 512])
            # 4 DoubleRow passes (K=256 each):
            #   P1 rows r-1/r+1, phase b=0: [WB(0) | WB(2)], t-stride 130
            #   P2 rows r-1/r+1, phase b=1: [WA(0) | WA(2)], t-stride 130
            #   P3 row  r,       phase b=0: [WB(1) | 0],     t-stride 0
            #   P4 row  r,       phase b=1: [WA(1) | 0],     t-stride 0
            passes = [
                (OFF + r0 * SW - 1, 130, 0),
                (OFF + r0 * SW, 130, 256),
                (OFF + (r0 + 1) * SW - 1, 0, 512),
                (OFF + (r0 + 1) * SW, 0, 768),
            ]
            mms = []
            for i, (b0, ts, w0) in enumerate(passes):
                sl = mega[:, b0 : b0 + 1]
                rap = bass.AP(
                    sl.tensor, sl.offset, [list(sl.ap[0]), [ts, 2], [1, N]]
                )
                mms.append(
                    nc.tensor.matmul(
                        ps[:, :],
                        wt_sb[:, w0 : w0 + 256].rearrange("k (t m) -> k t m", t=2),
                        rap,
                        start=(i == 0),
                        stop=(i == 3),
                        perf_mode=mybir.MatmulPerfMode.DoubleRow,
                    )
                )
            for a, b in zip(mms[1:], mms[:-1]):
                add_dep_helper(a.ins, b.ins, sync=False, reason="psum group order")

            # strip pads + cast into the batch's cm tile (split DVE / ACT)
            ps_v = ps[:].rearrange("p (s c) -> p s c", c=SW)[:, :, 1:65]
            out_ap = cm[:, cmoff : cmoff + g * 64].rearrange(
                "p (s c) -> p s c", c=64
            )
            if use_act:
                nc.scalar.activation(
                    out_ap, ps_v, mybir.ActivationFunctionType.Copy
                )
            else:
                nc.vector.tensor_copy(out=out_ap, in_=ps_v)

        # software-pipelined emission: per iteration, transpose/sign stage
        # tiles of image it interleaved with conv batches of image it-1 so
        # the PE stream never gaps; the next image's load is prefetched.
        xts = {}
        megas = {}
        xts[0] = input_load(0, chunks=first_load_chunks)
        megas[0] = mega_setup(0)

        def conv_image(img):
            mega = megas[img]
            out = []
            for batch in batches:
                rb0 = batch[0][0]
                RB = sum(g for _, g in batch)
                cm = cm_pool.tile([128, RB * 64], mybir.dt.float16)
                cmoff = 0
                items = []
                for r0, g in batch:
                    items.append((r0, g, cm, cmoff))
                    cmoff += g * 64
                out.append((cm, rb0, RB, items))
            return out

        for it in range(nimg + 1):
            flat = []  # (batch_idx, r0, g, cm, cmoff)
            work = []
            if it >= 1:
                work = conv_image(it - 1)
                for bi, (cm, rb0, RB, items) in enumerate(work):
                    for (r0, g, cm_, cmoff) in items:
                        flat.append((bi, r0, g, cm_, cmoff))
            has_t = it < nimg
            n_stages = 8 if has_t else 1
            per_stage = (len(flat) + n_stages - 1) // n_stages if flat else 0
            done_in_batch = {}
            gidx = 0
            fi = 0
            strip_set = last_act_strip if it == nimg else act_strip
            for q in range(n_stages):
                if has_t:
                    transpose_tile(it, megas[it], xts[it], q)
                    if q == 0 and it + 1 < nimg:
                        xts[it + 1] = input_load(it + 1, chunks=steady_chunks)
                        megas[it + 1] = mega_setup(it + 1)
                take = per_stage if q < n_stages - 1 else len(flat) - fi
                for _ in range(take):
                    if fi >= len(flat):
                        break
                    bi, r0, g, cm, cmoff = flat[fi]
                    conv_group(
                        it - 1, megas[it - 1], r0, g, cm, cmoff,
                        gidx in strip_set,
                    )
                    gidx += 1
                    done_in_batch[bi] = done_in_batch.get(bi, 0) + 1
                    if done_in_batch[bi] == len(work[bi][3]):
                        cm_t, rb0, RB, _ = work[bi]
                        nc.sync.dma_start(
                            out=y[it - 1, :, rb0 * 64 : (rb0 + RB) * 64],
                            in_=cm_t[:],
                        )
                    fi += 1
            if it >= 1:
                del megas[it - 1]
                del xts[it - 1]

    return nc


def build_nc_hybrid(nimg=NIMG, gsize=6, mega_bufs=3, psum_bufs=4, io_bufs=2,
                    tp_bufs=3, cm_bufs=4, gb=6, act_strip=(20, 21),
                    oct_act_strip=((0, 7), (1, 7)),
                    last_act_strip=((0, 1), (1, 1), (0, 3), (1, 3), (0, 5),
                                    (1, 5), (0, 7), (1, 7)),
                    first_load_chunks=4, oct_first=True, oct_last=True,
                    row_act_strip_last=tuple(range(0, 22, 2))):
    """Row-group steady pipeline with pair-octet convs for the first and
    last image.  Octet PSUM groups ([128, 64 slots x 8 pairs]) depend only
    on sign stage tiles {o, o+1} (stride-precise dep tracking; edges live
    in stage tile 0), so image 0's convs start during its own sign phase
    (saving a pipeline stage at startup) and image nimg-1's convs drain
    during its sign phase (cutting the tail)."""
    nc = bass.Bass()
    x = nc.dram_tensor("x", [nimg, H, W, C], mybir.dt.float32, kind="ExternalInput")
    wt = nc.dram_tensor("wt", [128, 1024], mybir.dt.float8e4, kind="ExternalInput")
    idents = nc.dram_tensor(
        "idents", [128, 128], mybir.dt.bfloat16, kind="ExternalInput"
    )
    y = nc.dram_tensor("y", [nimg, 128, H * C], mybir.dt.float16, kind="ExternalOutput")

    with tile.TileContext(nc) as tc, ExitStack() as ctx:
        wpool = ctx.enter_context(tc.tile_pool(name="wpool", bufs=1))
        mega_pool = ctx.enter_context(tc.tile_pool(name="mega", bufs=mega_bufs))
        in_pool = ctx.enter_context(tc.tile_pool(name="inp", bufs=io_bufs))
        cm_pool = ctx.enter_context(tc.tile_pool(name="cm", bufs=cm_bufs))
        tp_pool = ctx.enter_context(
            tc.tile_pool(name="tp", bufs=tp_bufs, space="PSUM")
        )
        psum_pool = ctx.enter_context(
            tc.tile_pool(name="ps", bufs=psum_bufs, space="PSUM")
        )

        wt_sb = wpool.tile([128, 1024], mybir.dt.float8e4)
        nc.sync.dma_start(out=wt_sb[:], in_=wt[:])
        id_sb = wpool.tile([128, 128], mybir.dt.bfloat16)
        nc.sync.dma_start(out=id_sb[:], in_=idents[:])

        def input_load(img, chunks=1):
            xt = in_pool.tile([128, H * 64], mybir.dt.bfloat16)
            wc = W // chunks
            # last chunk first: the edge transposes (stage tile 0) read
            # pixel 127 (last chunk) and pixel 0 (chunk 0)
            order = [chunks - 1] + list(range(chunks - 1)) if chunks > 1 else [0]
            for k in order:
                nc.gpsimd.dma_start(
                    out=xt[:, k * wc * 64 : (k + 1) * wc * 64],
                    in_=x[img, :, k * wc : (k + 1) * wc].rearrange(
                        "r w c -> r (w c)"
                    ),
                )
            return xt

        def mega_setup(img):
            mega = mega_pool.tile([128, MEGA_COLS], mybir.dt.float8e4)
            if img < mega_bufs:
                nc.gpsimd.memset(mega[:, 0:2], 0.0)
                slots = mega[:, OFF : OFF + 130 * SW].rearrange(
                    "p (s c) -> p s c", c=SW
                )
                nc.gpsimd.memset(slots[0:64, :, 0:1], 0.0)
                nc.gpsimd.memset(slots[64:128, :, 64:65], 0.0)
                nc.gpsimd.memset(mega[:, OFF : OFF + SW], 0.0)
                b128 = OFF + 129 * SW
                nc.gpsimd.memset(mega[:, b128 : b128 + SW], 0.0)
            return mega

        def transpose_tile(img, mega, xt, q):
            # stage tile q: q=0 holds the edge slot (odd@63 top / even@0
            # bottom) + blocks 0..6 (mega cols 1..7); q>=1 holds blocks
            # 8q-1..8q+6 (cols 8q..8q+7).  Octet o then needs only stage
            # tiles {o, o+1}.
            tp = tp_pool.tile([128, 1024], mybir.dt.bfloat16)
            if q == 0:
                nc.tensor.transpose(tp[0:64, 0:128], xt[:, 8128:8192], id_sb[:])
                nc.tensor.transpose(tp[64:128, 0:128], xt[:, 0:64], id_sb[:])
                blocks = range(0, 7)
                boff = 1  # block j sits at tp slot j+1
                c0 = 1
            else:
                blocks = range(8 * q - 1, 8 * q + 7)
                boff = -(8 * q - 1)
                c0 = 8 * q
            for g in blocks:
                j = g + boff
                nc.tensor.transpose(
                    tp[:, j * 128 : (j + 1) * 128],
                    xt[:, 64 + 128 * g : 192 + 128 * g],
                    id_sb[:],
                )
            if q == 0:
                # odd@63 -> top of col 64; even@0 -> bottom of col 0
                dsl2 = mega[0:64, OFF + SW + 64 : OFF + SW + 65]
                dst2 = bass.AP(dsl2.tensor, dsl2.offset, [list(dsl2.ap[0]), [SW, H]])
                ssl2 = tp[0:64, 0:1]
                src2 = bass.AP(ssl2.tensor, ssl2.offset, [list(ssl2.ap[0]), [1, H]])
                nc.scalar.activation(dst2, src2, mybir.ActivationFunctionType.Sign)
                dsl3 = mega[64:128, OFF + SW : OFF + SW + 1]
                dst3 = bass.AP(dsl3.tensor, dsl3.offset, [list(dsl3.ap[0]), [SW, H]])
                ssl3 = tp[64:128, 0:1]
                src3 = bass.AP(ssl3.tensor, ssl3.offset, [list(ssl3.ap[0]), [1, H]])
                nc.scalar.activation(dst3, src3, mybir.ActivationFunctionType.Sign)
                nblk, sslot = 7, 1
            else:
                nblk, sslot = 8, 0
            dsl = mega[:, OFF + SW + c0 : OFF + SW + c0 + 1]
            dst = bass.AP(
                dsl.tensor, dsl.offset, [list(dsl.ap[0]), [1, nblk], [SW, H]]
            )
            ssl = tp[:, sslot * 128 : sslot * 128 + 1]
            src = bass.AP(
                ssl.tensor, ssl.offset, [list(ssl.ap[0]), [128, nblk], [1, H]]
            )
            nc.scalar.activation(dst, src, mybir.ActivationFunctionType.Sign)

        def conv_octet(img, mega, h, o, cm, use_act):
            ps = psum_pool.tile([128, 512], mybir.dt.float32, name="psr")
            # bases at n=0 (slot 64h, pair 8o); rhs dims [p, t, slot, pair]
            passes = [
                (OFF + 64 * h * SW + 8 * o, 130, 0),
                (OFF + 64 * h * SW + 8 * o + 1, 130, 256),
                (OFF + (64 * h + 1) * SW + 8 * o, 0, 512),
                (OFF + (64 * h + 1) * SW + 8 * o + 1, 0, 768),
            ]
            mms = []
            for i, (b0, ts, w0) in enumerate(passes):
                sl = mega[:, b0 : b0 + 1]
                rap = bass.AP(
                    sl.tensor, sl.offset,
                    [list(sl.ap[0]), [ts, 2], [SW, 64], [1, 8]],
                )
                mms.append(
                    nc.tensor.matmul(
                        ps[:, :],
                        wt_sb[:, w0 : w0 + 256].rearrange("k (t m) -> k t m", t=2),
                        rap,
                        start=(i == 0),
                        stop=(i == 3),
                        perf_mode=mybir.MatmulPerfMode.DoubleRow,
                    )
                )
            for a, b in zip(mms[1:], mms[:-1]):
                add_dep_helper(a.ins, b.ins, sync=False, reason="psum group order")

            ps_v = ps[:].rearrange("p (s c) -> p s c", c=8)
            out_ap = cm[:].rearrange("p (s c) -> p s c", c=64)[:, :, 8 * o : 8 * o + 8]
            if use_act:
                nc.scalar.activation(
                    out_ap, ps_v, mybir.ActivationFunctionType.Copy
                )
            else:
                nc.vector.tensor_copy(out=out_ap, in_=ps_v)

        groups = []
        r0 = 0
        while r0 < H:
            g = min(gsize, H - r0)
            groups.append((r0, g))
            r0 += g
        GB = gb
        batches = [groups[i : i + GB] for i in range(0, len(groups), GB)]
        if len(batches) >= 2 and len(batches[-1]) < GB // 2:
            batches[-2].extend(batches.pop())

        def conv_group(img, mega, r0, g, cm, cmoff, use_act):
            N = g * SW
            ps = psum_pool.tile(
                [128, N], mybir.dt.float32, padded_shape=[128, 512], name="psr"
            )
            passes = [
                (OFF + r0 * SW - 1, 130, 0),
                (OFF + r0 * SW, 130, 256),
                (OFF + (r0 + 1) * SW - 1, 0, 512),
                (OFF + (r0 + 1) * SW, 0, 768),
            ]
            mms = []
            for i, (b0, ts, w0) in enumerate(passes):
                sl = mega[:, b0 : b0 + 1]
                rap = bass.AP(
                    sl.tensor, sl.offset, [list(sl.ap[0]), [ts, 2], [1, N]]
                )
                mms.append(
                    nc.tensor.matmul(
                        ps[:, :],
                        wt_sb[:, w0 : w0 + 256].rearrange("k (t m) -> k t m", t=2),
                        rap,
                        start=(i == 0),
                        stop=(i == 3),
                        perf_mode=mybir.MatmulPerfMode.DoubleRow,
                    )
                )
            for a, b in zip(mms[1:], mms[:-1]):
                add_dep_helper(a.ins, b.ins, sync=False, reason="psum group order")
            ps_v = ps[:].rearrange("p (s c) -> p s c", c=SW)[:, :, 1:65]
            out_ap = cm[:, cmoff : cmoff + g * 64].rearrange(
                "p (s c) -> p s c", c=64
            )
            if use_act:
                nc.scalar.activation(
                    out_ap, ps_v, mybir.ActivationFunctionType.Copy
                )
            else:
                nc.vector.tensor_copy(out=out_ap, in_=ps_v)

        xts = {}
        megas = {}
        cmss = {}
        xts[0] = input_load(0, chunks=first_load_chunks)
        megas[0] = mega_setup(0)

        def conv_img_octet(img, olist, strip_set):
            for o in olist:
                for h in range(2):
                    conv_octet(
                        img, megas[img], h, o, cmss[img][h],
                        (h, o) in strip_set,
                    )
                    if o == 7:
                        nc.sync.dma_start(
                            out=y[img, :, h * 4096 : (h + 1) * 4096],
                            in_=cmss[img][h][:],
                        )

        # iteration it: transposes/signs of image it; row-group convs of
        # image it-1 interleaved; octet convs for the first/last image run
        # in their own iteration (their octets unblock as sign tiles land).
        oct_imgs = set()
        if oct_first:
            oct_imgs.add(0)
        if oct_last:
            oct_imgs.add(nimg - 1)
        n_iters = nimg if oct_last else nimg + 1
        for it in range(n_iters):
            has_t = it < nimg
            if has_t:
                mega = megas[it]
                xt = xts[it]
                if it in oct_imgs:
                    cmss[it] = [
                        cm_pool.tile([128, 4096], mybir.dt.float16, name="cma"),
                        cm_pool.tile([128, 4096], mybir.dt.float16, name="cmb"),
                    ]
            prev = it - 1
            flat = []
            work = []
            if 0 <= prev < nimg and prev not in oct_imgs:
                for bi, batch in enumerate(batches):
                    rb0 = batch[0][0]
                    RB = sum(g for _, g in batch)
                    cm = cm_pool.tile(
                        [128, RB * 64], mybir.dt.float16, name="cmr"
                    )
                    cmoff = 0
                    items = []
                    for r0g, g in batch:
                        items.append((r0g, g, cm, cmoff))
                        cmoff += g * 64
                    work.append((cm, rb0, RB, items))
                for bi, (cm, rb0, RB, items) in enumerate(work):
                    for (r0g, g, cm_, cmoff) in items:
                        flat.append((bi, r0g, g, cm_, cmoff))
            row_strips = set(
                row_act_strip_last if prev == nimg - 1 else act_strip
            )
            per_stage = (len(flat) + 7) // 8 if flat else 0
            done_in_batch = {}
            gidx = 0
            fi = 0
            for q in range(8 if has_t else 1):
                if has_t:
                    transpose_tile(it, mega, xt, q)
                    if q == 0 and it + 1 < nimg:
                        xts[it + 1] = input_load(it + 1)
                        megas[it + 1] = mega_setup(it + 1)
                take = per_stage if (has_t and q < 7) else len(flat) - fi
                for _ in range(take):
                    if fi >= len(flat):
                        break
                    bi, r0g, g, cm, cmoff = flat[fi]
                    conv_group(
                        prev, megas[prev], r0g, g, cm, cmoff,
                        gidx in row_strips,
                    )
                    gidx += 1
                    done_in_batch[bi] = done_in_batch.get(bi, 0) + 1
                    if done_in_batch[bi] == len(work[bi][3]):
                        cm_t, rb0, RB, _ = work[bi]
                        nc.sync.dma_start(
                            out=y[prev, :, rb0 * 64 : (rb0 + RB) * 64],
                            in_=cm_t[:],
                        )
                    fi += 1
            if it == 0 and oct_first:
                conv_img_octet(0, list(range(8)), set(oct_act_strip))
            if has_t and it == nimg - 1 and oct_last:
                conv_img_octet(it, list(range(8)), set(last_act_strip))
            if it >= 2 and it - 2 in megas:
                del megas[it - 2]
            if it >= 1 and it - 1 in xts:
                del xts[it - 1]

    return nc


def make_wt(w_np):
    """Host-side weight prep: w [3,3,64,64] fp32 -> wt [128, 1024] fp8e4.

    Per-pass DoubleRow stacks [k, (t, m)]:
      cols   0:256  = [WB(0) | WB(2)]   (P1: rows r-1/r+1, phase b=0)
      cols 256:512  = [WA(0) | WA(2)]   (P2: rows r-1/r+1, phase b=1)
      cols 512:768  = [WB(1) | 0]       (P3: row r, phase b=0)
      cols 768:1024 = [WA(1) | 0]       (P4: row r, phase b=1)

    WA(dy) (rhs col c = pair j+1 = [odd@j | even@(j+1)]):
      [0:64,0:64]=ws[dy,2]; [0:64,64:]=ws[dy,1]; [64:,64:]=ws[dy,2]
    WB(dy) (rhs col c = pair j = [odd@(j-1) | even@j]):
      [0:64,0:64]=ws[dy,0]; [64:,0:64]=ws[dy,1]; [64:,64:]=ws[dy,0]
    M cols 0:64 = even-pixel outputs, 64:128 = odd-pixel outputs.
    """
    ws = np.sign(w_np).astype(np.float32)

    def WA(dy):
        m = np.zeros((128, 128), np.float32)
        m[0:64, 0:64] = ws[dy, 2]
        m[0:64, 64:128] = ws[dy, 1]
        m[64:128, 64:128] = ws[dy, 2]
        return m

    def WB(dy):
        m = np.zeros((128, 128), np.float32)
        m[0:64, 0:64] = ws[dy, 0]
        m[64:128, 0:64] = ws[dy, 1]
        m[64:128, 64:128] = ws[dy, 0]
        return m

    z = np.zeros((128, 128), np.float32)
    wt = np.concatenate(
        [WB(0), WB(2), WA(0), WA(2), WB(1), z, WA(1), z], axis=1
    )
    fp8 = mybir.dt.np(mybir.dt.float8e4)
    return wt.astype(fp8)


def make_idents():
    """[128, 128] bf16 identity (moving operand for PE transposes)."""
    return np.eye(128, dtype=np.float32).astype(ml_dtypes.bfloat16)


_NC_CACHE = {}


def get_nc():
    if "nc" not in _NC_CACHE:
        _NC_CACHE["nc"] = build_nc()
    return _NC_CACHE["nc"]


def kernel(x, w):
    from concourse.bass_utils import run_bass_kernel_spmd

    x = np.asarray(x, dtype=np.float32)
    w = np.asarray(w, dtype=np.float32)
    assert x.shape == (N_CORES * NIMG, H, W, C) and w.shape == (3, 3, C, C)

    wt = make_wt(w)
    idents = make_idents()
    nc = get_nc()
    in_maps = [
        {
            "x": np.ascontiguousarray(x[c * NIMG : (c + 1) * NIMG]),
            "wt": wt,
            "idents": idents,
        }
        for c in range(N_CORES)
    ]
    res = run_bass_kernel_spmd(nc, in_maps, list(range(N_CORES)))
    outs = []
    for c in range(N_CORES):
        ycm = np.asarray(res.results[c]["y"])  # [nimg, 128, H*64] fp16
        o = (
            ycm.reshape(NIMG, 2, 64, H, 64)  # (img, par, co, r, j)
            .transpose(0, 3, 4, 1, 2)        # (img, r, j, par, co)
            .reshape(NIMG, H, W, C)
            .astype(np.float32)
        )
        outs.append(o)
    return np.concatenate(outs, axis=0)
